# revision 54
# baseline (speedup 1.0000x reference)
"""GQA decode attention (B=16, S=4096, NH=32, NKV=8, HD=128) on 8 TRN2 cores.

Sharding: tensor-parallel over heads — 1 KV head (4 Q heads) per core.
Each core: qkv projection for its 768 wqkv rows, RoPE + QK-RMSNorm,
attention over its KV-head slice of the caches, RowParallel o_proj slice
producing a [16, 4096] partial; partials are summed on the host.

All large operands (x, wqkv, K cache, V cache, o_proj) are stored in HBM
as bf16 and repacked on the host so every DMA descriptor is a large
(2-8KB) per-partition contiguous run — per-queue DMA throughput is
descriptor-generation-limited, and small descriptors halve it.  The
harness tolerance (2e-2) has plenty of room for bf16 storage error
(~0.4%); PSUM accumulation stays f32 throughout.

Schedule (the kernel is jointly limited by HBM streaming at ~420 B/ns and
by the PE, which the power governor duty-throttles to ~50-60% while DMA is
hot — so the DMA streams are ordered by NEED and kept saturated):
 - sync queue: wq tiles (gate the whole qkv->attention chain) then the 16
   K^T tiles; vv/ow DMAs are WAR-gated behind the last wq tile so wq owns
   the early bandwidth.
 - scalar queue: o_proj weights, then the 16 V tiles (both gated).
 - nearly all wq tiles are SBUF-resident (wpool bufs=7) so the wq stream
   is never consumption-paced by the slow unramped early PE; the qkv
   accumulation is split even/odd across PSUM banks for 2-way PE ILP.
 - per-batch: scores = 32 chunk matmuls (K chunk stationary, 4 q-head
   columns moving), exp on the Act engine straight to bf16, AV = 32
   accumulating matmuls (exp chunk stationary, V chunk moving), then the
   batch is normalized and transposed immediately so the o_proj epilogue
   (weights preloaded, 4 batched output DMAs) only waits on the last batch.

The cache scatter at last_pos is handled by baking last_pos (host-known at
compile time, compile happens inside kernel()) into the program:
 - the stale cache position's softmax weight is zeroed via a rowmask
   multiply after the exp;
 - the true contribution e_new * [v_new, 1] is added back via a rank-1
   matmul into the attention accumulator.
Softmax skips max-subtraction (scores are ~N(0,1) after QK-RMSNorm); the
denominator is folded into the value matmul via a ones-column appended to V.
"""

import sys
from contextlib import ExitStack

for _p in ("/opt/trn_rl_repo",):
    if _p not in sys.path:
        sys.path.insert(0, _p)

import numpy as np

import concourse.bass as bass
import concourse.tile as tile
from concourse import mybir
from concourse.bass_utils import run_bass_kernel_spmd
from concourse.masks import make_identity

B, S, H = 16, 4096, 4096
NH, NKV, HD = 32, 8, 128
NREP = NH // NKV  # 4 q heads per kv head (= per core)
DQ = NREP * HD  # 512
NCORES = 8
EPS = 1e-5
NCH = S // 128  # 32 seq chunks
VW = 129  # V row width: 128 + 1 ones-column
F32 = mybir.dt.float32
BF16 = mybir.dt.bfloat16
AF = mybir.ActivationFunctionType
AX = mybir.AxisListType


def _legalize_waits(nc):
    """This walrus build accepts at most ONE sync wait on most instruction
    encodings (Matmult's S3_LW, DMA structs, ...) while Tile may attach
    several. Move excess waits onto same-engine no-ops inserted right before
    the instruction (semantically identical: the engine queue executes the
    wait no-ops, then the instruction)."""
    moved = 0
    skip = (mybir.InstNoOp, mybir.InstEventSemaphore)
    for func in nc.m.functions:
        for bb in func.blocks:
            insts = list(bb.instructions)
            out = []
            changed = False
            for inst in insts:
                si = inst.sync_info
                if (
                    si is not None
                    and si.on_wait
                    and len(si.on_wait) > 1
                    and not isinstance(inst, skip)
                ):
                    waits = list(si.on_wait)
                    for k, w in enumerate(waits[:-1]):
                        nop = mybir.InstNoOp(
                            name=f"{inst.name}-w{k}", engine=inst.engine
                        )
                        nop.sync_info = mybir.SyncInfo(on_wait=[w], on_update=[])
                        out.append(nop)
                        moved += 1
                    si.on_wait = waits[-1:]
                    inst.sync_info = si
                    changed = True
                out.append(inst)
            if changed:
                bb.instructions = out
    return moved


def _build_bass(lp, legalize=True, reps=1, kvbufs=6):
    """Build the SPMD Bass program. lp: tuple of 16 ints (last_pos, baked).

    reps > 1 repeats the whole computation (for slope-based timing: the
    per-call dispatch overhead cancels between two rep counts)."""
    nc = bass.Bass("TRN2", target_bir_lowering=False, debug=False)

    xt_d = nc.dram_tensor("xt", [128, NCH, B], BF16, kind="ExternalInput")
    # tile-major, partition-contiguous: one 4KB/2KB descriptor per partition
    wqa_d = nc.dram_tensor("wqa", [NCH // 4, 128, 4, DQ], BF16, kind="ExternalInput")
    wqb_d = nc.dram_tensor("wqb", [NCH // 4, 128, 4, 256], BF16, kind="ExternalInput")
    kt_d = nc.dram_tensor("kt", [B, 128, S], BF16, kind="ExternalInput")
    vv_d = nc.dram_tensor("vv", [B, 128, NCH, VW], BF16, kind="ExternalInput")
    ow_d = nc.dram_tensor("ow", [NREP, 128, 8, 512], BF16, kind="ExternalInput")
    cosq_d = nc.dram_tensor("cosq", [B, NREP, 64], F32, kind="ExternalInput")
    sinq_d = nc.dram_tensor("sinq", [B, NREP, 64], F32, kind="ExternalInput")
    cosk_d = nc.dram_tensor("cosk", [B, 64], F32, kind="ExternalInput")
    sink_d = nc.dram_tensor("sink", [B, 64], F32, kind="ExternalInput")
    rm_d = nc.dram_tensor("rowmask", [128, B], F32, kind="ExternalInput")
    out_d = nc.dram_tensor("out_p", [B, H], F32, kind="ExternalOutput")

    with tile.TileContext(nc) as tc, ExitStack() as ctx:
        consts = ctx.enter_context(tc.tile_pool(name="consts", bufs=1))
        sb = ctx.enter_context(tc.tile_pool(name="sb", bufs=2))
        kpool = ctx.enter_context(tc.tile_pool(name="kpool", bufs=kvbufs))
        vpool = ctx.enter_context(tc.tile_pool(name="vpool", bufs=4))
        # nearly all wqa/wqb tiles resident: the wq stream is never
        # consumption-paced by the (slow, unramped) early PE
        wpool = ctx.enter_context(tc.tile_pool(name="wpool", bufs=7))

        ident = consts.tile([128, 128], F32)
        make_identity(nc, ident[:, :])

        xt_sb = consts.tile([128, NCH, B], BF16)
        nc.sync.dma_start(out=xt_sb[:, :, :], in_=xt_d[:, :, :])
        cosq = consts.tile([B, NREP, 64], F32)
        sinq = consts.tile([B, NREP, 64], F32)
        cosk = consts.tile([B, 64], F32)
        sink = consts.tile([B, 64], F32)
        epsq = consts.tile([B, 1], F32)
        epsk = consts.tile([B, 1], F32)
        nc.vector.memset(epsq[:, :], float(HD * EPS))
        nc.vector.memset(epsk[:, :], float(EPS))
        nc.sync.dma_start(out=cosq[:, :, :], in_=cosq_d[:, :, :])
        nc.sync.dma_start(out=sinq[:, :, :], in_=sinq_d[:, :, :])
        nc.sync.dma_start(out=cosk[:, :], in_=cosk_d[:, :])
        nc.sync.dma_start(out=sink[:, :], in_=sink_d[:, :])
        rowmask = consts.tile([128, B], F32)
        nc.sync.dma_start(out=rowmask[:, :], in_=rm_d[:, :])

        for rep in range(reps):
            qn = consts.tile([B, NREP, 64, 2], F32)  # rope'd+normed q (with 1/sqrt(HD))
            kn = consts.tile([B, HD], F32)  # rope'd+normed k
            vn = consts.tile([B, VW], BF16)  # new v row: [v_new, 1]
            enew = consts.tile([B, NREP], BF16)  # exp(q . k_new / sqrt(HD))
            qT_bf = consts.tile([128, B * NREP], BF16)  # col b*4+h
            oT_sb = consts.tile([128, NREP, B], BF16)  # attention out, [d, (g, b)]

            # ---- qkv projection: qkv[b, o] = sum_h x[b, h] * wqkv_c[o, h] ----
            # Even/odd chunks accumulate into separate PSUM banks (2-way ILP on
            # the PE instead of one serial accumulation chain), combined by DVE.
            qkv_ps_ctx = tc.tile_pool(name="psq", bufs=1, space="PSUM")
            psq = qkv_ps_ctx.__enter__()
            ps_qA = psq.tile([B, NREP * 128], F32)
            ps_qB = psq.tile([B, NREP * 128], F32)
            ps_kvA = psq.tile([B, 256], F32)
            ps_kvB = psq.tile([B, 256], F32)
            for ii in range(NCH // 4):
                wt = wpool.tile([128, 4, DQ], BF16, tag="wqa")
                nc.sync.dma_start(out=wt[:, :, :], in_=wqa_d[ii, :, :, :])
                for k in range(4):
                    i = 4 * ii + k
                    nc.tensor.matmul(
                        ps_qA if i % 2 == 0 else ps_qB,
                        xt_sb[:, i, :], wt[:, k, :],
                        start=(i < 2), stop=(i >= NCH - 2),
                    )
            wq_gate = None
            for ii in range(NCH // 4):
                wt = wpool.tile([128, 4, 256], BF16, tag="wqb")
                nc.sync.dma_start(out=wt[:, :, :], in_=wqb_d[ii, :, :, :])
                if ii == NCH // 4 - 1:
                    wq_gate = wt
                for k in range(4):
                    i = 4 * ii + k
                    nc.tensor.matmul(
                        ps_kvA if i % 2 == 0 else ps_kvB,
                        xt_sb[:, i, :], wt[:, k, :],
                        start=(i < 2), stop=(i >= NCH - 2),
                    )
            qsum = sb.tile([B, NREP, 64, 2], F32, tag="qsum")
            kvsum = sb.tile([B, 2, 64, 2], F32, tag="kvsum")
            qsum_f = qsum[:, :, :, :].rearrange("p a b c -> p (a b c)")
            kvsum_f = kvsum[:, :, :, :].rearrange("p a b c -> p (a b c)")
            nc.vector.tensor_copy(qsum_f, ps_qA)
            nc.vector.tensor_add(qsum_f, qsum_f, ps_qB)
            nc.vector.tensor_copy(kvsum_f, ps_kvA)
            nc.vector.tensor_add(kvsum_f, kvsum_f, ps_kvB)
            qkv_ps_ctx.__exit__(None, None, None)
            q_ev, q_od = qsum[:, :, :, 0], qsum[:, :, :, 1]
            k_ev, k_od = kvsum[:, 0, :, 0], kvsum[:, 0, :, 1]
            v_new = kvsum[:, 1, :, :].rearrange("p a b -> p (a b)")

            # ---- RoPE (interleaved pairs) + QK-RMSNorm, all in [B, .] layout ----
            t0 = sb.tile([B, NREP, 64], F32, tag="t0")
            t1 = sb.tile([B, NREP, 64], F32, tag="t1")
            nc.vector.tensor_mul(t0[:, :, :], q_ev, cosq[:, :, :])
            nc.vector.tensor_mul(t1[:, :, :], q_od, sinq[:, :, :])
            nc.vector.tensor_sub(qn[:, :, :, 0], t0[:, :, :], t1[:, :, :])
            nc.vector.tensor_mul(t0[:, :, :], q_od, cosq[:, :, :])
            nc.vector.tensor_mul(t1[:, :, :], q_ev, sinq[:, :, :])
            nc.vector.tensor_add(qn[:, :, :, 1], t0[:, :, :], t1[:, :, :])

            kn2 = kn[:, :].rearrange("p (a b) -> p a b", b=2)
            t2 = sb.tile([B, 64], F32, tag="t2")
            t3 = sb.tile([B, 64], F32, tag="t3")
            nc.vector.tensor_mul(t2[:, :], k_ev, cosk[:, :])
            nc.vector.tensor_mul(t3[:, :], k_od, sink[:, :])
            nc.vector.tensor_sub(kn2[:, :, 0], t2[:, :], t3[:, :])
            nc.vector.tensor_mul(t2[:, :], k_od, cosk[:, :])
            nc.vector.tensor_mul(t3[:, :], k_ev, sink[:, :])
            nc.vector.tensor_add(kn2[:, :, 1], t2[:, :], t3[:, :])

            # new v row with ones-column (v has no rope/norm)
            nc.vector.tensor_copy(vn[:, 0:HD], v_new)
            nc.vector.memset(vn[:, HD:VW], 1.0)

            # RMSNorm q; fold in the 1/sqrt(HD) score scale:
            # rstd' = 1/sqrt(ssq + HD*eps) = rsqrt(mean(q^2)+eps)/sqrt(HD)
            qn128 = qn[:, :, :, :].rearrange("p a b c -> p a (b c)")  # [16, 4, 128]
            sq = sb.tile([B, NREP, HD], F32, tag="sq")
            nc.vector.tensor_mul(sq[:, :, :], qn128, qn128)
            ssq = sb.tile([B, NREP, 1], F32, tag="ssq")
            nc.vector.reduce_sum(out=ssq[:, :, :], in_=sq[:, :, :], axis=AX.X)
            rstdq = sb.tile([B, NREP, 1], F32, tag="rstdq")
            nc.scalar.activation(rstdq[:, :, :], ssq[:, :, :], AF.Sqrt, bias=epsq[:, :])
            nc.vector.reciprocal(rstdq[:, :, :], rstdq[:, :, :])
            for h in range(NREP):
                nc.vector.tensor_scalar_mul(qn128[:, h, :], qn128[:, h, :], rstdq[:, h, :])

            # RMSNorm k (no extra scale)
            sk = sb.tile([B, HD], F32, tag="sk")
            nc.vector.tensor_mul(sk[:, :], kn[:, :], kn[:, :])
            ssk = sb.tile([B, 1], F32, tag="ssk")
            nc.vector.reduce_sum(out=ssk[:, :], in_=sk[:, :], axis=AX.X)
            nc.scalar.activation(ssk[:, :], ssk[:, :], AF.Sqrt, scale=1.0 / HD, bias=epsk[:, :])
            nc.vector.reciprocal(ssk[:, :], ssk[:, :])
            nc.vector.tensor_scalar_mul(kn[:, :], kn[:, :], ssk[:, :])

            # e_new[b, h] = exp(qn . kn)  (scale already folded into qn)
            prod = sb.tile([B, NREP, HD], F32, tag="prod")
            kb = kn[:, :].unsqueeze(1).broadcast_to((B, NREP, HD))
            nc.vector.tensor_mul(prod[:, :, :], qn128, kb)
            snew = sb.tile([B, NREP, 1], F32, tag="snew")
            nc.vector.reduce_sum(out=snew[:, :, :], in_=prod[:, :, :], axis=AX.X)
            nc.scalar.activation(enew[:, :].unsqueeze(2), snew[:, :, :], AF.Exp)

            # ---- transpose q to [HD, .] layout via PE; cast to bf16 ----
            with tc.tile_pool(name="psT", bufs=1, space="PSUM") as psT:
                ps_qT = psT.tile([128, NREP * B], F32)  # col h*16+b
                for h in range(NREP):
                    nc.tensor.transpose(
                        ps_qT[:, h * B:(h + 1) * B],
                        qn128[:, h, :],
                        ident[0:B, 0:B],
                    )
                # reorder h*16+b -> b*4+h while copying to SBUF (casts to bf16)
                qT_src = ps_qT[:, :].rearrange("p (h b) -> p b h", h=NREP)
                qT_dst = qT_bf[:, :].rearrange("p (b h) -> p b h", h=NREP)
                nc.vector.tensor_copy(qT_dst, qT_src)

            # ---- attention over the streamed caches ----
            # DMA need-order: wq (above) then kt on the sync queue; vv and ow
            # on the scalar queue, both WAR-gated behind the last wq tile so
            # wq owns the early bandwidth.  The two queues carry ~equal bytes
            # (kt 16.8MB vs vv+ow 21.1MB); the scheduler fills the scalar
            # queue's consumption-paced gaps with the ow pieces, which are
            # only needed by the o_proj epilogue.
            # All cache DMAs are issued up-front (in batch order); the pool
            # buffer ring (kvbufs deep) paces the queues automatically.
            ow_all = consts.tile([128, NREP, 8, 512], BF16, tag="owall")
            for g in range(NREP):
                nc.vector.tensor_scalar_mul(
                    ow_all[0:1, g, 0, 0:1], wq_gate[0:1, 0, 0:1], 0.0
                )
                nc.scalar.dma_start(out=ow_all[:, g, :, :], in_=ow_d[g, :, :, :])
            kts, vvs = [], []
            for b in range(B):
                kt_t = kpool.tile([128, S], BF16, tag="kt")
                nc.sync.dma_start(out=kt_t[:, :], in_=kt_d[b, :, :])
                kts.append(kt_t)
                vv_t = vpool.tile([128, NCH, VW], BF16, tag="vv")
                if b < 4:
                    # WAR gate: hold the vv stream's first tiles until the wq
                    # stream has fully landed, so wq (which gates the whole
                    # qkv -> attention chain) owns the early HBM bandwidth.
                    nc.vector.tensor_scalar_mul(
                        vv_t[0:1, 0, 0:1], wq_gate[0:1, 0, 0:1], 0.0
                    )
                nc.scalar.dma_start(out=vv_t[:, :, :], in_=vv_d[b, :, :, :])
                vvs.append(vv_t)

            # per-batch e_new row masks (only row b alive), off the critical path
            enew_all = consts.tile([B, B, NREP], BF16, tag="enewall")
            for b in range(B):
                nc.vector.tensor_scalar_mul(
                    enew_all[:, b, :], enew[:, :], ident[0:B, b:b + 1]
                )

            acc_sb = consts.tile([NREP, B, VW], F32)
            attn_n = consts.tile([NREP, B, HD], F32)
            with (
                tc.tile_pool(name="psc", bufs=3, space="PSUM") as psc_pool,
                tc.tile_pool(name="pso", bufs=3, space="PSUM") as pso_pool,
                tc.tile_pool(name="psT2", bufs=1, space="PSUM") as psT2,
            ):
                ps_oT = psT2.tile([128, B * NREP], F32)  # col b*4+h
                for b in range(B):
                    pb = lp[b]
                    jb, rb = pb // 128, pb % 128
                    kt_t = kts[b]
                    vv_t = vvs[b]

                    psc = psc_pool.tile([128, 128], F32, tag="psc")
                    for j in range(NCH):
                        nc.tensor.matmul(
                            psc[:, 4 * j:4 * j + 4],
                            kt_t[:, 128 * j:128 * (j + 1)],
                            qT_bf[:, 4 * b:4 * b + 4],
                            start=True, stop=True,
                        )
                    expt = sb.tile([128, 128], BF16, tag="expt", bufs=3)
                    nc.scalar.activation(expt[:, :], psc[:, :], AF.Exp)
                    # scatter at last_pos: zero the stale position's weight; its
                    # true contribution e_new * [v_new, 1] is added back via the
                    # masked rank-1 matmuls below.
                    nc.vector.tensor_scalar_mul(
                        expt[:, 4 * jb:4 * jb + 4],
                        expt[:, 4 * jb:4 * jb + 4],
                        rowmask[:, b:b + 1],
                    )
                    pso = pso_pool.tile([NREP, VW], F32, tag="pso")
                    for j in range(NCH):
                        nc.tensor.matmul(
                            pso[:, :],
                            expt[:, 4 * j:4 * j + 4],
                            vv_t[:, j, :],
                            start=(j == 0), stop=False,
                        )
                    nc.tensor.matmul(
                        pso[:, :], enew_all[:, b, :], vn[:, :],
                        start=False, stop=True,
                    )
                    nc.vector.tensor_copy(acc_sb[:, b, :], pso[:, :])

                    # normalize + transpose this batch right away so the
                    # o_proj epilogue only waits on the last batch
                    rec = sb.tile([NREP, 1], F32, tag="rec")
                    nc.vector.reciprocal(rec[:, :], acc_sb[:, b, HD:HD + 1])
                    nc.vector.tensor_scalar_mul(
                        attn_n[:, b, :], acc_sb[:, b, 0:HD], rec[:, 0:1]
                    )
                    nc.tensor.transpose(
                        ps_oT[:, 4 * b:4 * b + 4],
                        attn_n[:, b, :],
                        ident[0:NREP, 0:NREP],
                    )

                oT_src = ps_oT[:, :].rearrange("p (b h) -> p h b", h=NREP)
                nc.vector.tensor_copy(oT_sb[:, :, :], oT_src)

            # ---- o_proj: out[b, :] = sum_g oT[:, g, b] . ow[g] ----
            with (
                tc.tile_pool(name="psO", bufs=2, space="PSUM") as psO,
            ):
                for half in range(4):
                    out_sb = sb.tile([B, 2, 512], F32, tag="outsb")
                    for k in range(2):
                        nb = 2 * half + k
                        ps_out = psO.tile([B, 512], F32, tag="po")
                        for g in range(NREP):
                            nc.tensor.matmul(
                                ps_out[:, :], oT_sb[:, g, :], ow_all[:, g, nb, :],
                                start=(g == 0), stop=(g == NREP - 1),
                            )
                        nc.vector.tensor_copy(out_sb[:, k, :], ps_out[:, :])
                    nc.sync.dma_start(
                        out=out_d[:, 1024 * half:1024 * (half + 1)],
                        in_=out_sb[:, :, :].rearrange("p a b -> p (a b)"),
                    )

    if legalize:
        _legalize_waits(nc)
    return nc


def _prep_inputs(x, last_pos, rope_cache, wqkv, o_proj_w, cache_k, cache_v):
    import ml_dtypes
    f32 = np.float32
    bf16 = ml_dtypes.bfloat16
    x2 = np.asarray(x, f32).reshape(B, H)
    lp = tuple(int(v) for v in np.asarray(last_pos).reshape(-1))
    rc = np.asarray(rope_cache, f32)[list(lp)]  # [16, 64, 2]
    cos, sin = rc[..., 0].copy(), rc[..., 1].copy()  # [16, 64]
    cosq = np.ascontiguousarray(np.broadcast_to(cos[:, None, :], (B, NREP, 64)))
    sinq = np.ascontiguousarray(np.broadcast_to(sin[:, None, :], (B, NREP, 64)))

    xt = np.ascontiguousarray(x2.T.reshape(NCH, 128, B).transpose(1, 0, 2)).astype(bf16)

    wqkv = np.asarray(wqkv, f32)
    o_proj_w = np.asarray(o_proj_w, f32)
    cache_k = np.asarray(cache_k, f32)
    cache_v = np.asarray(cache_v, f32)

    # [8, 16, 128, 4096] : per-core K^T, bf16
    ktall = np.ascontiguousarray(cache_k.transpose(2, 0, 3, 1)).astype(bf16)
    # [8, 16, 128, 32, 129] : per-core V with ones column, chunk-major repack
    v5 = cache_v.reshape(B, NCH, 128, NKV, HD).transpose(3, 0, 2, 1, 4)
    vvall = np.zeros((NKV, B, 128, NCH, VW), bf16)
    vvall[..., :HD] = v5.astype(bf16)
    vvall[..., HD] = 1.0

    rowmask = np.ones((128, B), f32)
    for b in range(B):
        rowmask[lp[b] % 128, b] = 0.0

    per_core = []
    for c in range(NCORES):
        w_c = np.concatenate(
            [
                wqkv[c * DQ:(c + 1) * DQ],
                wqkv[NH * HD + c * HD:NH * HD + (c + 1) * HD],
                wqkv[NH * HD + NKV * HD + c * HD:NH * HD + NKV * HD + (c + 1) * HD],
            ],
            axis=0,
        )  # [768, 4096]
        # [H, 768] -> tile-major [NCH//4, 128, 4, 768]: h = 512*ii + 128*k + p
        wqT = np.ascontiguousarray(w_c.T).reshape(NCH // 4, 4, 128, 768)
        wqT = wqT.transpose(0, 2, 1, 3)  # [ii, p, k, o]
        wqa_c = np.ascontiguousarray(wqT[..., 0:DQ]).astype(bf16)
        wqb_c = np.ascontiguousarray(wqT[..., DQ:768]).astype(bf16)
        ow_c = np.ascontiguousarray(o_proj_w[:, c * DQ:(c + 1) * DQ].T).reshape(
            NREP, 128, 8, 512
        ).astype(bf16)
        per_core.append(
            {
                "xt": xt,
                "wqa": wqa_c,
                "wqb": wqb_c,
                "kt": ktall[c],
                "vv": vvall[c],
                "ow": ow_c,
                "cosq": cosq,
                "sinq": sinq,
                "cosk": cos,
                "sink": sin,
                "rowmask": rowmask,
            }
        )
    return lp, per_core


_NC_CACHE = {}
LAST_RESULT = None  # BassKernelResults of the most recent run (for profiling)


def kernel(**inputs):
    x = inputs["x"]
    last_pos = inputs["last_pos"]
    lp, per_core = _prep_inputs(
        x,
        last_pos,
        inputs["rope_cache"],
        inputs["wqkv"],
        inputs["o_proj_w"],
        inputs["cache_k"],
        inputs["cache_v"],
    )
    if lp not in _NC_CACHE:
        _NC_CACHE[lp] = _build_bass(lp)
    nc = _NC_CACHE[lp]
    res = run_bass_kernel_spmd(nc, per_core, core_ids=list(range(NCORES)))
    global LAST_RESULT
    LAST_RESULT = res
    results = res.results if hasattr(res, "results") else res
    out = np.zeros((B, H), np.float64)
    for c in range(NCORES):
        out += results[c]["out_p"].astype(np.float64)
    return out.astype(np.float32).reshape(B, 1, H)
